# revision 3
# baseline (speedup 1.0000x reference)
"""DEDICOM decoder kernel for 8 Trainium2 NeuronCores.

Math (per relation k, K=8):
    score[k, i] = sigmoid( (r_i * d_k) @ G @ (d_k * c_i) )
                = sigmoid( sum_ab r_ia * Wk[a,b] * c_ib ),   Wk = diag(d_k) G diag(d_k)

Sharding: data-parallel over rows (N=500000) across 8 cores; Wk replicated.

Per-core dataflow (row block BL=512, layout [feature-partition, row-free]):
  - host pre-transposes R, C to [D, N] so DMA loads are contiguous
  - PE:  Y_k^T = Wk^T @ R^T           (matmul, f32r: full-rate fp32 path)
  - DVE: P_k   = Y_k^T * C^T          (elementwise, PSUM x SBUF -> SBUF)
  - PE:  S    += E_k^T @ P_k          (ones-column matmul = partition-dim
                                       reduction; accumulates all K rows
                                       into one [K, BL] PSUM tile)
  - ACT: out   = sigmoid(S)           -> DMA to HBM as [K, N] (final layout)
"""

import sys

sys.path.insert(0, "/opt/trn_rl_repo")

import os
from contextlib import ExitStack

import numpy as np

import concourse.bass as bass
import concourse.tile as tile
from concourse import bacc, mybir
from concourse.bass_utils import run_bass_kernel_spmd

N, D, K, NCORES = 500000, 128, 8, 8
BL = 512
NPC = N // NCORES              # 62500 rows per core
NB = (NPC + BL - 1) // BL      # 123 blocks
NPAD = NB * BL                 # 62976 padded rows per core
F32 = mybir.dt.float32
F32R = mybir.dt.float32r

# number of relation slices whose elementwise product runs on GpSimd
# (sourced from an SBUF copy made by ScalarE) instead of VectorE
GPSIMD_K = int(os.environ.get("DEDICOM_GPSIMD_K", "0"))

_NC_CACHE = {}

LAST_RESULTS = None  # BassKernelResults of the most recent run (for test.py)


def build_bass(npad=NPAD):
    nb = npad // BL
    nc = bacc.Bacc(
        "TRN2", target_bir_lowering=False, debug=False, enable_asserts=False
    )
    rt = nc.dram_tensor("rt", [D, npad], F32R, kind="ExternalInput").ap()
    ct = nc.dram_tensor("ct", [D, npad], F32, kind="ExternalInput").ap()
    wk = nc.dram_tensor("wk", [D, K * D], F32R, kind="ExternalInput").ap()
    em = nc.dram_tensor("em", [K, D, K], F32R, kind="ExternalInput").ap()
    out = nc.dram_tensor("out", [K, npad], F32, kind="ExternalOutput").ap()

    with tile.TileContext(nc) as tc, ExitStack() as ctx:
        const_pool = ctx.enter_context(tc.tile_pool(name="const", bufs=1))
        in_pool = ctx.enter_context(tc.tile_pool(name="inp", bufs=6))
        p_pool = ctx.enter_context(tc.tile_pool(name="prod", bufs=6))
        y_pool = ctx.enter_context(tc.tile_pool(name="ypsum", bufs=4, space="PSUM"))
        s_pool = ctx.enter_context(tc.tile_pool(name="spsum", bufs=2, space="PSUM"))
        ysb_pool = ctx.enter_context(tc.tile_pool(name="ysb", bufs=4))
        o_pool = ctx.enter_context(tc.tile_pool(name="outp", bufs=4))

        w_sb = const_pool.tile([D, K * D], F32R, tag="w")
        nc.sync.dma_start(w_sb[:], wk[:])
        e_sb = []
        for k in range(K):
            e = const_pool.tile([D, K], F32R, tag=f"em{k}")
            nc.sync.dma_start(e[:], em[k])
            e_sb.append(e)

        for j in range(nb):
            sl = bass.ts(j, BL)
            rt_t = in_pool.tile([D, BL], F32R, tag="rt")
            nc.sync.dma_start(rt_t[:], rt[:, sl])
            ct_t = in_pool.tile([D, BL], F32, tag="ct")
            nc.sync.dma_start(ct_t[:], ct[:, sl])

            ps_s = s_pool.tile([K, BL], F32, tag="s")
            for k in range(K):
                ps_y = y_pool.tile([D, BL], F32, tag="y")
                nc.tensor.matmul(
                    ps_y[:],
                    lhsT=w_sb[:, bass.ts(k, D)],
                    rhs=rt_t[:],
                    start=True,
                    stop=True,
                )
                p_t = p_pool.tile([D, BL], F32R, tag="p")
                if k >= K - GPSIMD_K:
                    # route through SBUF so GpSimd (no PSUM access) can help
                    y_sb = ysb_pool.tile([D, BL], F32, tag="ysb")
                    nc.scalar.copy(y_sb[:], ps_y[:])
                    nc.gpsimd.tensor_tensor(
                        p_t[:], y_sb[:], ct_t[:], mybir.AluOpType.mult
                    )
                else:
                    nc.vector.tensor_tensor(
                        p_t[:], ps_y[:], ct_t[:], mybir.AluOpType.mult
                    )
                nc.tensor.matmul(
                    ps_s[:],
                    lhsT=e_sb[k][:],
                    rhs=p_t[:],
                    start=(k == 0),
                    stop=(k == K - 1),
                )
            sig = o_pool.tile([K, BL], F32, tag="sig")
            nc.scalar.activation(
                sig[:], ps_s[:], mybir.ActivationFunctionType.Sigmoid
            )
            nc.sync.dma_start(out[:, sl], sig[:])
    nc.compile()
    return nc


def _get_nc(npad=NPAD):
    if npad not in _NC_CACHE:
        _NC_CACHE[npad] = build_bass(npad)
    return _NC_CACHE[npad]


def _host_inputs(inputs_row, inputs_col, global_interaction, local_variation):
    R = np.asarray(inputs_row, dtype=np.float32)
    C = np.asarray(inputs_col, dtype=np.float32)
    G = np.asarray(global_interaction, dtype=np.float64)
    lv = np.asarray(local_variation, dtype=np.float64)

    # Wk[a,b] = d_k[a] * G[a,b] * d_k[b], laid out [a, (k,b)]
    w = lv[:, :, None] * G[None, :, :] * lv[:, None, :]        # [K, a, b]
    wk = np.ascontiguousarray(
        np.transpose(w, (1, 0, 2)).reshape(D, K * D)
    ).astype(np.float32)

    em = np.zeros((K, D, K), dtype=np.float32)
    for k in range(K):
        em[k, :, k] = 1.0

    rt_full = np.ascontiguousarray(R.T)   # [D, N]
    ct_full = np.ascontiguousarray(C.T)

    in_maps = []
    for c in range(NCORES):
        rt_c = np.zeros((D, NPAD), dtype=np.float32)
        rt_c[:, :NPC] = rt_full[:, c * NPC : (c + 1) * NPC]
        ct_c = np.zeros((D, NPAD), dtype=np.float32)
        ct_c[:, :NPC] = ct_full[:, c * NPC : (c + 1) * NPC]
        in_maps.append({"rt": rt_c, "ct": ct_c, "wk": wk, "em": em})
    return in_maps


def kernel(inputs_row, inputs_col, global_interaction, local_variation):
    global LAST_RESULTS
    in_maps = _host_inputs(
        inputs_row, inputs_col, global_interaction, local_variation
    )
    nc = _get_nc()
    trace = os.environ.get("DEDICOM_TRACE", "0") == "1"
    try:
        res = run_bass_kernel_spmd(
            nc, in_maps, list(range(NCORES)), trace=trace
        )
    except Exception:
        if not trace:
            raise
        res = run_bass_kernel_spmd(nc, in_maps, list(range(NCORES)))
    LAST_RESULTS = res
    out = np.empty((K, N), dtype=np.float32)
    for c in range(NCORES):
        out[:, c * NPC : (c + 1) * NPC] = res.results[c]["out"][:, :NPC]
    return out
